# revision 11
# baseline (speedup 1.0000x reference)
"""Trainium2 Bass kernel for the CDGR gnn_message_passing module.

Mathematically exact reformulation of the reference:

  - softmax rows of A sum to 1  =>  L = I - A, the d-scaling vanishes
  - s2l logits are additively separable in (pixel, node) => the softmax
    over pixels is identical for every node column => app collapses to a
    rank-1 outer product relu(G) (x) softmax(w_in . x)
  - the semantic branch (word attention + 2-layer GCN) is batch
    independent => computed once per core (in bf16; it only feeds the
    rank-1 app term and is well inside the 2e-2 tolerance)
  - the two chained 1x1 convs fuse: Wlg = final_w[:, :C] @ gw_w.  The
    fwT rows feeding Wlg are negated host-side so the spiral can be
    produced as -spiral = EXV/D - xv in a single fused vector op.
  - the `+ x` residual is folded into the final matmul as an
    identity-weight accumulation

Per batch (2 per core, data-parallel over 8 cores):
  out[o,q] = relu( Wlg @ spiral^T + fa (x) ea + x )  with
  spiral = xv - (E @ xv) / D,  E = exp(S - ub),  S = x_phi @ Dg @ x_phi_T
  computed via S^T tiles (lhsT = M_ext columns) so that E^T column
  slices feed the big E @ xv matmul directly as lhsT, with a fused ones
  column in xv giving D, and a fused K=17 row giving the -ub shift.

Schedule notes (TimelineSim-guided):
  - weights ship in three host-packed DRAM images: packP (tiny, phi/glob/
    win/adj -- unblocks the per-batch preamble immediately), packH
    (gww/final_w), packS (bf16 semantic weights)
  - the per-batch R spill/reload (the torch .view raw-reshape) is the
    critical path, so bulk loads that are needed later carry explicit
    order deps pushing them behind it on the serial DMA device
  - a dozen dummy identity transposes warm the PE p-state during the
    first x DMA
  - the lone Sqrt (adjacency norm) runs at t~2us so its activation-table
    load never interrupts the exp stream
"""

import os
from contextlib import ExitStack

import numpy as np

import concourse.bass as bass
import concourse.bacc as bacc
import concourse.mybir as mybir
import concourse.tile as tile
from concourse import masks
from concourse.bass_utils import run_bass_kernel_spmd
from concourse.tile_rust import add_dep_helper

FP = mybir.dt.float32
BF = mybir.dt.bfloat16
FR = mybir.dt.float32r
AF = mybir.ActivationFunctionType
ALU = mybir.AluOpType
AX = mybir.AxisListType

NCORES = 8
BPC = 2          # batches per core
C, HW = 256, 1024
MPHI, NN, DE = 16, 20, 300
KE = DE + 1      # 301 = DEMB + fused-bias row

LAST_EXEC_NS = None
LAST_RESULT = None


def _ksl(total, step=128):
    return [(o, min(step, total - o)) for o in range(0, total, step)]


def _fr(ap):
    return ap.bitcast(FR)


# ---------------------------------------------------------------------------
# weight-pack layouts (shared between host packing and kernel build)
# ---------------------------------------------------------------------------

class _PackAlloc:
    """First-fit strip allocator: blocks of equal width stack vertically in a
    128-row strip before opening a new column range."""

    def __init__(self):
        self.strips = []            # [col_off, width, used_rows]
        self.ncols = 0
        self.blocks = {}            # name -> (row, col, rows, cols)

    def add(self, name, rows, cols, stack=False):
        # PE matmul operands must sit at base partition 0 (they pair with
        # base-0 tiles); only non-matmul blocks may stack below other blocks.
        if stack:
            for s in self.strips:
                r = (s[2] + 31) // 32 * 32
                if s[1] == cols and r <= 64 and r + rows <= 128:
                    s[2] = r + rows
                    self.blocks[name] = (r, s[0], rows, cols)
                    return
        off = self.ncols
        self.ncols += cols
        self.strips.append([off, cols, rows])
        self.blocks[name] = (0, off, rows, cols)


def _mk_layout_p():
    a = _PackAlloc()
    for i in range(2):
        a.add(f"phiwT{i}", 128, MPHI)
    for i in range(2):
        a.add(f"globwT{i}", 128, MPHI)
    for i in range(2):
        a.add(f"win{i}", 128, 1)
    a.add("phib", MPHI, 1, stack=True)
    a.add("adj", NN, NN)
    return a


def _mk_layout_h():
    a = _PackAlloc()
    for i in range(2):
        a.add(f"gww{i}", 128, C)
    for i in range(4):
        a.add(f"fwT{i}", 128, C)
    return a


def _mk_layout_s():
    a = _PackAlloc()
    for nm, k in (("wq", DE), ("wk", DE), ("wve", KE), ("wo", DE)):
        for i, (o, s) in enumerate(_ksl(k)):
            a.add(f"{nm}{i}", s, DE)
    for i, (o, s) in enumerate(_ksl(DE)):
        a.add(f"gc1{i}", s, C)
    for i in range(2):
        a.add(f"gc2{i}", 128, C)
    for i, (o, s) in enumerate(_ksl(KE)):
        a.add(f"embTe{i}", s, NN)
    a.add("emb", NN, DE, stack=True)
    a.add("bo", 1, DE, stack=True)
    for nm, k in (("bq", DE), ("bk", DE)):
        for i, (o, s) in enumerate(_ksl(k)):
            a.add(f"{nm}{i}", s, 1, stack=True)
    return a


_LP = _mk_layout_p()
_LH = _mk_layout_h()
_LS = _mk_layout_s()


def _pack_p(inputs):
    f = lambda k: np.ascontiguousarray(inputs[k], dtype=np.float32)
    img = np.zeros((128, _LP.ncols), np.float32)

    def put(name, arr):
        r, c, rows, cols = _LP.blocks[name]
        img[r:r + rows, c:c + cols] = arr

    phiwT = f("phi_w").T
    globwT = f("glob_w").T
    for i, (o, s) in enumerate(_ksl(C)):
        put(f"phiwT{i}", phiwT[o:o + s])
        put(f"globwT{i}", globwT[o:o + s])
        put(f"win{i}", f("s2l_w")[:C].reshape(C, 1)[o:o + s])
    put("phib", f("phi_b").reshape(MPHI, 1))
    put("adj", f("adj"))
    return img


def _pack_h(inputs):
    f = lambda k: np.ascontiguousarray(inputs[k], dtype=np.float32)
    img = np.zeros((128, _LH.ncols), np.float32)

    def put(name, arr):
        r, c, rows, cols = _LH.blocks[name]
        img[r:r + rows, c:c + cols] = arr

    for i, (o, s) in enumerate(_ksl(C)):
        put(f"gww{i}", f("gw_w")[o:o + s])
    fwT = f("final_w").T
    for i, (o, s) in enumerate(_ksl(2 * C)):
        # rows 0:256 (the Wlg half) are negated: the kernel computes
        # -spiral^T, and (-Wlg) @ (-spiral^T) = Wlg @ spiral^T.
        blk = fwT[o:o + s]
        put(f"fwT{i}", -blk if i < 2 else blk)
    return img


def _pack_s(inputs):
    bf = mybir.dt.np(BF)
    f = lambda k: np.ascontiguousarray(inputs[k], dtype=np.float32)
    img = np.zeros((128, _LS.ncols), bf)

    def put(name, arr):
        r, c, rows, cols = _LS.blocks[name]
        img[r:r + rows, c:c + cols] = arr.astype(bf)

    wve = np.vstack([f("wv"), f("bv")[None, :]])
    embTe = np.vstack([f("emb").T, np.ones((1, NN), np.float32)])
    for nm, k, arr in (("wq", DE, f("wq")), ("wk", DE, f("wk")),
                       ("wve", KE, wve), ("wo", DE, f("wo")),
                       ("gc1", DE, f("gc1_w")), ("embTe", KE, embTe),
                       ("bq", DE, f("bq").reshape(DE, 1)),
                       ("bk", DE, f("bk").reshape(DE, 1))):
        for i, (o, s) in enumerate(_ksl(k)):
            put(f"{nm}{i}", arr[o:o + s])
    for i, (o, s) in enumerate(_ksl(C)):
        put(f"gc2{i}", f("gc2_w")[o:o + s])
    put("emb", f("emb"))
    put("bo", f("bo").reshape(1, DE))
    return img


# ---------------------------------------------------------------------------
# kernel build
# ---------------------------------------------------------------------------

def _build_nc():
    nc = bacc.Bacc()

    x_p = nc.declare_dram_parameter("x", [BPC, C * HW], FP, isOutput=False)
    out_p = nc.declare_dram_parameter("out", [BPC, C * HW], FP, isOutput=True)
    pp_p = nc.declare_dram_parameter("wpackP", [128, _LP.ncols], FP,
                                     isOutput=False)
    ph_p = nc.declare_dram_parameter("wpackH", [128, _LH.ncols], FP,
                                     isOutput=False)
    ps_p = nc.declare_dram_parameter("wpackS", [128, _LS.ncols], BF,
                                     isOutput=False)
    rscr = nc.dram_tensor("rscratch", [BPC, MPHI * HW], FP)

    with tile.TileContext(nc) as tc:
        with nc.allow_low_precision(reason="float32r/bf16 matmul feeds"), \
             ExitStack() as ctx:
            _body(ctx, tc, nc, x_p, out_p, pp_p, ph_p, ps_p, rscr)
    nc.finalize()
    return nc


def _body(ctx, tc, nc, x_p, out_p, pp_p, ph_p, ps_p, rscr):
    cw = ctx.enter_context(tc.tile_pool(name="cw", bufs=1))      # persistent
    sem = ctx.enter_context(tc.tile_pool(name="sem", bufs=1))    # semantic
    sm = ctx.enter_context(tc.tile_pool(name="sm", bufs=2))      # small/batch
    xm = ctx.enter_context(tc.tile_pool(name="xm", bufs=2))
    xvp = ctx.enter_context(tc.tile_pool(name="xvp", bufs=2))
    rp = ctx.enter_context(tc.tile_pool(name="rp", bufs=2))
    etp = ctx.enter_context(tc.tile_pool(name="etp", bufs=16))
    spp = ctx.enter_context(tc.tile_pool(name="spp", bufs=2))
    obp = ctx.enter_context(tc.tile_pool(name="obp", bufs=4))
    ps_w = ctx.enter_context(tc.tile_pool(name="ps_w", bufs=2, space="PSUM"))
    ps_x = ctx.enter_context(tc.tile_pool(name="ps_x", bufs=3, space="PSUM"))
    ps_t = ctx.enter_context(tc.tile_pool(name="ps_t", bufs=1, space="PSUM"))

    def mm(out, lhsT, rhs, start, stop):
        nc.tensor.matmul(out, _fr(lhsT), _fr(rhs), start=start, stop=stop)

    def mmb(out, lhsT, rhs, start, stop):
        nc.tensor.matmul(out, lhsT, rhs, start=start, stop=stop)

    # ---------------- constants + PE warmup + first loads ----------------
    ident = cw.tile([128, 128], FP, tag="ident")
    masks.make_identity(nc, ident[:])

    # p-state warmup: dummy transposes keep the PE continuously busy while
    # the first DMAs land, so real matmuls start at full clock.
    for i in range(12):
        wu = ps_t.tile([128, 128], FP, tag="ps_t", name=f"wu{i}")
        nc.tensor.transpose(_fr(wu[:]), _fr(ident[:]), _fr(ident[:]))

    packP = cw.tile([128, _LP.ncols], FP, tag="packP")
    nc.sync.dma_start(_fr(packP[:]), _fr(pp_p[:]))

    def wp(name):
        r, c, rows, cols = _LP.blocks[name]
        return packP[r:r + rows, c:c + cols]

    packH = cw.tile([128, _LH.ncols], FP, tag="packH")
    packS = cw.tile([128, _LS.ncols], BF, tag="packS")

    def wh(name):
        r, c, rows, cols = _LH.blocks[name]
        return packH[r:r + rows, c:c + cols]

    def ws(name):
        r, c, rows, cols = _LS.blocks[name]
        return packS[r:r + rows, c:c + cols]

    Me = [cw.tile([MPHI + 1, HW], FP, tag=f"Me{b}", name=f"Me{b}")
          for b in range(BPC)]
    for b in range(BPC):
        nc.gpsimd.memset(Me[b][MPHI:MPHI + 1, :], 1.0)
    one_row_b = cw.tile([1, NN], BF, tag="one_row_b")
    nc.gpsimd.memset(one_row_b[:], 1.0)
    ones20b = cw.tile([NN, 8], BF, tag="ones20b")
    nc.gpsimd.memset(ones20b[:], 1.0)
    inv20b = cw.tile([NN, 8], BF, tag="inv20b")
    nc.gpsimd.memset(inv20b[:], 1.0 / NN)

    x_mat = x_p[:].rearrange("b (c2 c q) -> b c c2 q", c2=2, c=128, q=HW)
    x_raw = x_p[:].rearrange("b (t q c) -> b q t c", t=8, q=128, c=C)
    out_v = out_p[:].rearrange("b (o2 o q) -> b o o2 q", o2=2, o=128, q=HW)
    r_st = rscr[:].rearrange("b (j q) -> b j q", j=MPHI)
    r_ld = rscr[:].rearrange("b (t p m) -> b p t m", t=8, p=128, m=MPHI)

    xmat = [None] * BPC
    xvt = [None] * BPC
    xvv = [None] * BPC

    def load_xmat(b):
        t = xm.tile([128, 2 * HW], FP, tag="xmat", name=f"xmat{b}")
        d = nc.sync.dma_start(_fr(t[:].rearrange("c (c2 q) -> c c2 q", c2=2)),
                              _fr(x_mat[b]))
        xmat[b] = t
        return d

    def load_xv(b):
        t = xvp.tile([128, 8 * (C + 1)], FP, tag="xv", name=f"xv{b}")
        v = t[:].rearrange("q (t c) -> q t c", t=8)
        d = nc.sync.dma_start(_fr(v[:, :, 0:C]), _fr(x_raw[b]))
        nc.gpsimd.memset(v[:, :, C:C + 1], 1.0)
        xvt[b] = t
        xvv[b] = v
        return d

    d_xmat0 = load_xmat(0)

    # adjacency normalisation early: its Sqrt table load must not land in
    # the middle of the exp stream.  adj_n = (d (x) d) * (adj + I)
    ah = sem.tile([NN, NN], FP, tag="ah")
    nc.gpsimd.tensor_add(ah[:], wp("adj"), ident[0:NN, 0:NN])
    r20 = sem.tile([NN, 1], FP, tag="r20")
    nc.vector.tensor_reduce(r20[:], ah[:], axis=AX.X, op=ALU.add)
    ir20 = sem.tile([NN, 1], FP, tag="ir20")
    nc.vector.reciprocal(ir20[:], r20[:])
    d20 = sem.tile([NN, 1], FP, tag="d20")
    nc.scalar.activation(d20[:], ir20[:], AF.Sqrt)
    ps = ps_t.tile([1, NN], FP, tag="ps_t")
    nc.tensor.transpose(ps[:], d20[:, 0:1], ident[0:NN, 0:NN])
    dT = sem.tile([1, NN], FP, tag="dT")
    nc.vector.tensor_copy(_fr(dT[:]), ps[:])
    ps = ps_t.tile([NN, NN], FP, tag="ps_t")
    mm(ps[:], dT[:], dT[:], start=True, stop=True)
    adjn = sem.tile([NN, NN], FP, tag="adjn")
    nc.vector.tensor_mul(adjn[:], ah[:], ps[:])
    ps = ps_t.tile([NN, NN], FP, tag="ps_t")
    nc.tensor.transpose(ps[:], adjn[:], ident[0:NN, 0:NN])
    adjnT = sem.tile([NN, NN], BF, tag="adjnT")
    nc.vector.tensor_copy(adjnT[:], ps[:])

    # ---------------- per-batch phases ----------------
    R = [None] * BPC
    negMm = [None] * BPC
    xpT = [None] * BPC
    ea = [None] * BPC
    fab = [None] * BPC
    ETb = [None] * BPC

    def phase_pre(b):
        """phi -> R (+spill), Dg, M rows, -rowmax."""
        xmv = xmat[b][:].rearrange("c (c2 q) -> c c2 q", c2=2)
        pphi = ps_w.tile([MPHI, HW], FP, tag="ps_w")
        for ki in range(2):
            for nh in range(2):
                mm(pphi[:, 512 * nh:512 * (nh + 1)], wp(f"phiwT{ki}"),
                   xmv[:, ki, 512 * nh:512 * (nh + 1)],
                   start=(ki == 0), stop=(ki == 1))
        Rb = rp.tile([MPHI, HW], FP, tag="R")
        nc.scalar.activation(_fr(Rb[:]), pphi[:], AF.Relu,
                             bias=wp("phib")[:, 0:1])
        d_rst = nc.gpsimd.dma_start(r_st[b], Rb[:])
        R[b] = Rb

        xmean = sm.tile([128, 2], FP, tag="xmean")
        for ki in range(2):
            nc.vector.tensor_reduce(xmean[:, ki:ki + 1], xmv[:, ki, :],
                                    axis=AX.X, op=ALU.add)
        pg = ps_t.tile([MPHI, 1], FP, tag="ps_t")
        for ki in range(2):
            mm(pg[:], wp(f"globwT{ki}"), xmean[:, ki:ki + 1],
               start=(ki == 0), stop=(ki == 1))
        # sigmoid via exp+recip (keeps the act table on the main set)
        eng = sm.tile([MPHI, 1], FP, tag="eng")
        nc.scalar.activation(eng[:], pg[:, 0:1], AF.Exp,
                             scale=float(-1.0 / HW))
        nc.gpsimd.tensor_scalar_add(eng[:], eng[:], 1.0)
        sm05 = sm.tile([MPHI, 1], FP, tag="sm05")
        nc.vector.reciprocal(sm05[:], eng[:])
        nc.gpsimd.tensor_scalar_add(sm05[:], sm05[:], -0.5)
        Dgb = sm.tile([MPHI, MPHI], FP, tag="Dg")
        nc.gpsimd.tensor_scalar(_fr(Dgb[:]), ident[0:MPHI, 0:MPHI],
                                sm05[:, 0:1], 0.5, op0=ALU.mult, op1=ALU.add)

        pm = ps_w.tile([MPHI, HW], FP, tag="ps_w")
        for nh in range(2):
            mm(pm[:, 512 * nh:512 * (nh + 1)], Dgb[:],
               Rb[:, 512 * nh:512 * (nh + 1)], start=True, stop=True)
        nc.vector.tensor_copy(_fr(Me[b][0:MPHI, :]), pm[:])
        Mmax = sm.tile([MPHI, 1], FP, tag="Mmax")
        nc.vector.tensor_reduce(Mmax[:], pm[:], axis=AX.X, op=ALU.max)
        nMm = sm.tile([MPHI, 1], FP, tag="negMm")
        nc.vector.tensor_scalar_mul(_fr(nMm[:]), Mmax[:], -1.0)
        negMm[b] = nMm
        return d_rst

    def phase_xpt(b):
        """x_phi reload (raw reshape), PE transposes, ub row."""
        xpa = sm.tile([128, 128], FP, tag="xpa")
        d_xpl = nc.gpsimd.dma_start(
            xpa[:].rearrange("p (t m) -> p t m", t=8), r_ld[b])
        xt = rp.tile([MPHI + 1, HW], FP, tag="xpT")
        for h in range(2):
            psx = ps_x.tile([MPHI, 512], FP, tag="ps_x")
            for j in range(4):
                t8 = 4 * h + j
                nc.tensor.transpose(_fr(psx[:, 128 * j:128 * (j + 1)]),
                                    _fr(xpa[:, MPHI * t8:MPHI * (t8 + 1)]),
                                    _fr(ident[:, :]))
            nc.vector.tensor_copy(_fr(xt[0:MPHI, 512 * h:512 * (h + 1)]),
                                  psx[:])
        pub = ps_w.tile([1, HW], FP, tag="ps_w")
        for nh in range(2):
            mm(pub[:, 512 * nh:512 * (nh + 1)], negMm[b][:, 0:1],
               xt[0:MPHI, 512 * nh:512 * (nh + 1)], start=True, stop=True)
        # nub row split across ACT and DVE (it sits on the critical path)
        nc.scalar.copy(_fr(xt[MPHI:MPHI + 1, 0:512]), pub[:, 0:512])
        nc.vector.tensor_copy(_fr(xt[MPHI:MPHI + 1, 512:HW]), pub[:, 512:HW])
        xpT[b] = xt
        return d_xpl

    def phase_ea(b):
        """softmax-over-pixels numerator (normalisation folded into fa)."""
        xmv = xmat[b][:].rearrange("c (c2 q) -> c c2 q", c2=2)
        pa = ps_w.tile([1, HW], FP, tag="ps_w")
        for ki in range(2):
            for nh in range(2):
                mm(pa[:, 512 * nh:512 * (nh + 1)], wp(f"win{ki}"),
                   xmv[:, ki, 512 * nh:512 * (nh + 1)],
                   start=(ki == 0), stop=(ki == 1))
        eab = rp.tile([1, HW], FP, tag="ea")
        sae = sm.tile([1, 1], FP, tag="sae")
        nc.scalar.activation(eab[:], pa[:], AF.Exp, accum_out=sae[:, 0:1])
        sar = sm.tile([1, 1], FP, tag="sar", name=f"sar{b}")
        nc.vector.reciprocal(sar[:], sae[:])
        ea[b] = eab
        return sar

    def phase_et(b):
        """E^T tiles [128, 1024] = exp(S^T - ub)."""
        ET = []
        for t8 in range(8):
            pst = ps_w.tile([128, HW], FP, tag="ps_w")
            for nh in range(2):
                mm(pst[:, 512 * nh:512 * (nh + 1)],
                   Me[b][:, 128 * t8:128 * (t8 + 1)],
                   xpT[b][:, 512 * nh:512 * (nh + 1)], start=True, stop=True)
            et = etp.tile([128, HW], FP, tag="et")
            nc.scalar.activation(_fr(et[:]), pst[:], AF.Exp)
            ET.append(et)
        ETb[b] = ET

    def phase_exv(b):
        """EXV per p-tile; -spiral; transpose into -spiral^T."""
        ET = ETb[b]
        spT = spp.tile([128, 2 * HW], FP, tag="spT")
        spTv = spT[:].rearrange("c (ch p) -> c ch p", ch=2)
        for pt in range(8):
            pe = ps_x.tile([128, C + 1], FP, tag="ps_x")
            for k in range(8):
                mm(pe[:], ET[k][:, 128 * pt:128 * (pt + 1)],
                   xvt[b][:, 257 * k:257 * k + 257],
                   start=(k == 0), stop=(k == 7))
            rd = sm.tile([128, 1], FP, tag="rd")
            nc.vector.reciprocal(rd[:], pe[:, C:C + 1])
            spr = sm.tile([128, C], FP, tag="spr")
            nc.vector.scalar_tensor_tensor(spr[:], pe[:, 0:C], rd[:, 0:1],
                                           xvv[b][:, pt, 0:C],
                                           op0=ALU.mult, op1=ALU.subtract)
            ptr = ps_x.tile([128, C], FP, tag="ps_x")
            for ch in range(2):
                nc.tensor.transpose(_fr(ptr[:, 128 * ch:128 * (ch + 1)]),
                                    _fr(spr[:, 128 * ch:128 * (ch + 1)]),
                                    _fr(ident[:, :]))
            nc.vector.tensor_copy(
                _fr(spTv[:, :, 128 * pt:128 * (pt + 1)]),
                ptr[:].rearrange("p (ch q) -> p ch q", ch=2))
        return spTv

    def phase_out(b, spTv, WlgT):
        xmv = xmat[b][:].rearrange("c (c2 q) -> c c2 q", c2=2)
        for ot in range(2):
            po = ps_w.tile([128, HW], FP, tag="ps_w")
            for nh in range(2):
                sl = slice(512 * nh, 512 * (nh + 1))
                for ct in range(2):
                    mm(po[:, sl], WlgT[ct][:, 128 * ot:128 * (ot + 1)],
                       spTv[:, ct, sl], start=(ct == 0), stop=False)
                mm(po[:, sl], fab[b][0:1, 128 * ot:128 * (ot + 1)],
                   ea[b][0:1, sl], start=False, stop=False)
                mm(po[:, sl], ident[:, :], xmv[:, ot, sl],
                   start=False, stop=True)
            for h in range(2):
                ob = obp.tile([128, 512], FP, tag="ob")
                nc.scalar.activation(_fr(ob[:]), po[:, 512 * h:512 * (h + 1)],
                                     AF.Relu)
                nc.sync.dma_start(
                    _fr(out_v[b][:, ot, 512 * h:512 * (h + 1)]), _fr(ob[:]))

    # ---------------- emission: batch 0 front, staged loads ----------------
    phase_pre(0)
    d_xp0 = phase_xpt(0)
    sar0 = phase_ea(0)
    phase_et(0)

    # bulk loads pushed behind the batch-0 R roundtrip on the DMA device
    d_xv0 = load_xv(0)
    add_dep_helper(d_xp0.ins, d_xv0.ins, sync=True, reason="dma order")
    d_xmat1 = load_xmat(1)
    add_dep_helper(d_xp0.ins, d_xmat1.ins, sync=True, reason="dma order")
    d_packS = nc.sync.dma_start(packS[:], ps_p[:])
    add_dep_helper(d_xmat1.ins, d_packS.ins, sync=True, reason="dma order")

    phase_pre(1)
    d_xp1 = phase_xpt(1)
    sar1 = phase_ea(1)
    phase_et(1)

    d_xv1 = load_xv(1)
    add_dep_helper(d_xp1.ins, d_xv1.ins, sync=True, reason="dma order")
    d_packH = nc.sync.dma_start(_fr(packH[:]), _fr(ph_p[:]))
    add_dep_helper(d_xp1.ins, d_packH.ins, sync=True, reason="dma order")

    # ---------------- semantic branch (batch independent, bf16) ----------
    def qt_like(wname, bname, tag):
        outs = []
        for mi, (mo, ms) in enumerate(_ksl(DE)):
            ps = ps_t.tile([ms, NN], FP, tag="ps_t")
            for ki, (ko, ks) in enumerate(_ksl(DE)):
                mmb(ps[:], ws(f"{wname}{ki}")[:, mo:mo + ms],
                    ws(f"embTe{ki}")[0:ks, :], start=(ki == 0), stop=(ki == 2))
            t = sem.tile([ms, NN], BF, tag=f"{tag}{mi}")
            nc.scalar.activation(t[:], ps[:], AF.Identity,
                                 bias=ws(f"{bname}{mi}")[:, 0:1])
            outs.append(t)
        return outs

    qT = qt_like("wq", "bq", "qT")
    kT = qt_like("wk", "bk", "kT")

    ps = ps_t.tile([NN, DE], FP, tag="ps_t")
    for ki in range(3):
        mmb(ps[:], ws(f"embTe{ki}"), ws(f"wve{ki}"),
            start=(ki == 0), stop=(ki == 2))
    v_sb = sem.tile([NN, DE], BF, tag="v_sb")
    nc.vector.tensor_copy(v_sb[:], ps[:])

    ps = ps_t.tile([NN, NN], FP, tag="ps_t")
    for ki in range(3):
        mmb(ps[:], qT[ki][:], kT[ki][:], start=(ki == 0), stop=(ki == 2))
    att_s = sem.tile([NN, NN], FP, tag="att_s")
    nc.scalar.activation(att_s[:], ps[:], AF.Identity,
                         scale=float(1.0 / np.sqrt(DE)))
    mx = sem.tile([NN, 1], FP, tag="mx")
    nc.vector.tensor_reduce(mx[:], att_s[:], axis=AX.X, op=ALU.max)
    negmx = sem.tile([NN, 1], FP, tag="negmx")
    nc.vector.tensor_scalar_mul(negmx[:], mx[:], -1.0)
    att_e = sem.tile([NN, NN], FP, tag="att_e")
    rs = sem.tile([NN, 1], FP, tag="rs")
    nc.scalar.activation(att_e[:], att_s[:], AF.Exp, bias=negmx[:, 0:1],
                         accum_out=rs[:, 0:1])
    rr = sem.tile([NN, 1], FP, tag="rr")
    nc.vector.reciprocal(rr[:], rs[:])
    att_n = sem.tile([NN, NN], FP, tag="att_n")
    nc.vector.tensor_scalar_mul(att_n[:], att_e[:], rr[:, 0:1])

    ps = ps_t.tile([NN, NN], FP, tag="ps_t")
    nc.tensor.transpose(ps[:], att_n[:], ident[0:NN, 0:NN])
    attT = sem.tile([NN, NN], BF, tag="attT")
    nc.vector.tensor_copy(attT[:], ps[:])
    ps = ps_t.tile([NN, DE], FP, tag="ps_t")
    mmb(ps[:], attT[:], v_sb[:], start=True, stop=True)
    av_sb = sem.tile([NN, DE], BF, tag="av_sb")
    nc.vector.tensor_copy(av_sb[:], ps[:])

    n1c = sem.tile([128, 3], BF, tag="n1c")
    for mi, (mo, ms) in enumerate(_ksl(DE)):
        ps = ps_t.tile([ms, 8], FP, tag="ps_t")
        mmb(ps[:], av_sb[:, mo:mo + ms], inv20b[:], start=True, stop=True)
        nc.vector.tensor_copy(n1c[0:ms, mi:mi + 1], ps[:, 0:1])

    ps = ps_t.tile([1, DE], FP, tag="ps_t")
    for ki, (ko, ks) in enumerate(_ksl(DE)):
        mmb(ps[:], n1c[0:ks, ki:ki + 1], ws(f"wo{ki}"),
            start=(ki == 0), stop=(ki == 2))
    n2 = sem.tile([1, DE], BF, tag="n2")
    nc.vector.tensor_add(n2[:], ws("bo"), ps[:])
    ps = ps_t.tile([NN, DE], FP, tag="ps_t")
    mmb(ps[:], one_row_b[:], n2[:], start=True, stop=True)
    ev_sb = sem.tile([NN, DE], FP, tag="ev_sb")
    nc.vector.tensor_add(ev_sb[:], ws("emb"), ps[:])

    evT = []
    for mi, (mo, ms) in enumerate(_ksl(DE)):
        ps = ps_t.tile([ms, NN], FP, tag="ps_t")
        nc.tensor.transpose(ps[:], ev_sb[:, mo:mo + ms], ident[0:NN, 0:NN])
        t = sem.tile([ms, NN], BF, tag=f"evT{mi}")
        nc.vector.tensor_copy(t[:], ps[:])
        evT.append(t)

    ps = ps_t.tile([NN, C], FP, tag="ps_t")
    for ki in range(3):
        mmb(ps[:], evT[ki][:], ws(f"gc1{ki}"), start=(ki == 0), stop=(ki == 2))
    t1 = sem.tile([NN, C], BF, tag="t1")
    nc.vector.tensor_copy(t1[:], ps[:])
    ps = ps_t.tile([NN, C], FP, tag="ps_t")
    mmb(ps[:], adjnT[:], t1[:], start=True, stop=True)
    g1 = sem.tile([NN, C], FP, tag="g1")
    nc.vector.tensor_scalar_max(g1[:], ps[:], 0.0)

    g1T = []
    for mi, (mo, ms) in enumerate(_ksl(C)):
        ps = ps_t.tile([ms, NN], FP, tag="ps_t")
        nc.tensor.transpose(ps[:], g1[:, mo:mo + ms], ident[0:NN, 0:NN])
        t = sem.tile([ms, NN], BF, tag=f"g1T{mi}")
        nc.vector.tensor_copy(t[:], ps[:])
        g1T.append(t)

    ps = ps_t.tile([NN, C], FP, tag="ps_t")
    for ki in range(2):
        mmb(ps[:], g1T[ki][:], ws(f"gc2{ki}"), start=(ki == 0), stop=(ki == 1))
    t2 = sem.tile([NN, C], BF, tag="t2")
    nc.vector.tensor_copy(t2[:], ps[:])
    ps = ps_t.tile([NN, C], FP, tag="ps_t")
    mmb(ps[:], adjnT[:], t2[:], start=True, stop=True)
    g2 = sem.tile([NN, C], BF, tag="g2")
    nc.vector.tensor_scalar_max(g2[:], ps[:], 0.0)

    reluG = sem.tile([128, 2], FP, tag="reluG")
    for cb in range(2):
        ps = ps_t.tile([128, 8], FP, tag="ps_t")
        mmb(ps[:], g2[:, 128 * cb:128 * (cb + 1)], ones20b[:],
            start=True, stop=True)
        nc.scalar.activation(_fr(reluG[:, cb:cb + 1]), ps[:, 0:1], AF.Relu)
    ps = ps_t.tile([1, C], FP, tag="ps_t")
    for cb in range(2):
        mm(ps[:], reluG[:, cb:cb + 1], wh(f"fwT{2 + cb}"),
           start=(cb == 0), stop=(cb == 1))
    fa = sem.tile([1, C], FP, tag="fa")
    nc.vector.tensor_copy(_fr(fa[:]), ps[:])

    WlgT = []
    for cb in range(2):
        ps = ps_t.tile([128, C], FP, tag="ps_t")
        for ki in range(2):
            mm(ps[:], wh(f"gww{ki}")[:, 128 * cb:128 * (cb + 1)],
               wh(f"fwT{ki}"), start=(ki == 0), stop=(ki == 1))
        t = sem.tile([128, C], FP, tag=f"WlgT{cb}")
        nc.vector.tensor_copy(_fr(t[:]), ps[:])
        WlgT.append(t)

    for b, sar in ((0, sar0), (1, sar1)):
        fb = sm.tile([1, C], FP, tag="fab", name=f"fab{b}")
        nc.vector.tensor_scalar_mul(_fr(fb[:]), fa[:], sar[0:1, 0:1])
        fab[b] = fb

    # ---------------- heavy tail: EXV + out per batch ----------------
    spTv0 = phase_exv(0)
    phase_out(0, spTv0, WlgT)
    spTv1 = phase_exv(1)
    phase_out(1, spTv1, WlgT)


# ---------------------------------------------------------------------------
# host driver
# ---------------------------------------------------------------------------

def _prep_shared(inputs):
    return {"wpackP": _pack_p(inputs), "wpackH": _pack_h(inputs),
            "wpackS": _pack_s(inputs)}


_NC_CACHE = {}


def kernel(**inputs):
    global LAST_EXEC_NS, LAST_RESULT
    if "nc" not in _NC_CACHE:
        _NC_CACHE["nc"] = _build_nc()
    nc = _NC_CACHE["nc"]

    x = np.ascontiguousarray(inputs["x"], dtype=np.float32)
    B = x.shape[0]
    shared = _prep_shared(inputs)
    in_maps = []
    for i in range(NCORES):
        m = dict(shared)
        m["x"] = np.ascontiguousarray(
            x[i * BPC:(i + 1) * BPC].reshape(BPC, C * HW))
        in_maps.append(m)

    trace = os.environ.get("KERNEL_TRACE", "0") == "1"
    res = run_bass_kernel_spmd(nc, in_maps, list(range(NCORES)), trace=trace)
    LAST_RESULT = res
    LAST_EXEC_NS = getattr(res, "exec_time_ns", None)

    out = np.empty((B, C, 32, 32), np.float32)
    for i in range(NCORES):
        out[i * BPC:(i + 1) * BPC] = res.results[i]["out"].reshape(BPC, C, 32, 32)
    return out


# revision 12
# speedup vs baseline: 1.0643x; 1.0643x over previous
"""Trainium2 Bass kernel for the CDGR gnn_message_passing module.

Mathematically exact reformulation of the reference:

  - softmax rows of A sum to 1  =>  L = I - A, the d-scaling vanishes
  - s2l logits are additively separable in (pixel, node) => the softmax
    over pixels is identical for every node column => app collapses to a
    rank-1 outer product relu(G) (x) softmax(w_in . x)
  - the semantic branch (word attention + 2-layer GCN) is batch
    independent => computed once per core (in bf16; it only feeds the
    rank-1 app term and is well inside the 2e-2 tolerance)
  - the two chained 1x1 convs fuse: Wlg = final_w[:, :C] @ gw_w.  The
    fwT rows feeding Wlg are negated host-side so the spiral can be
    produced as -spiral = EXV/D - xv in a single fused vector op.
  - the `+ x` residual is folded into the final matmul as an
    identity-weight accumulation

Per batch (2 per core, data-parallel over 8 cores):
  out[o,q] = relu( Wlg @ spiral^T + fa (x) ea + x )  with
  spiral = xv - (E @ xv) / D,  E = exp(S - ub),  S = x_phi @ Dg @ x_phi_T
  computed via S^T tiles (lhsT = M_ext columns) so that E^T column
  slices feed the big E @ xv matmul directly as lhsT, with a fused ones
  column in xv giving D, and a fused K=17 row giving the -ub shift.

Schedule notes (TimelineSim-guided):
  - weights ship in three host-packed DRAM images: packP (tiny, phi/glob/
    win/adj -- unblocks the per-batch preamble immediately), packH
    (gww/final_w), packS (bf16 semantic weights)
  - the per-batch R spill/reload (the torch .view raw-reshape) is the
    critical path, so bulk loads that are needed later carry explicit
    order deps pushing them behind it on the serial DMA device
  - a dozen dummy identity transposes warm the PE p-state during the
    first x DMA
  - the lone Sqrt (adjacency norm) runs at t~2us so its activation-table
    load never interrupts the exp stream
"""

import os
from contextlib import ExitStack

import numpy as np

import concourse.bass as bass
import concourse.bacc as bacc
import concourse.mybir as mybir
import concourse.tile as tile
from concourse import masks
from concourse.bass_utils import run_bass_kernel_spmd
from concourse.tile_rust import add_dep_helper

FP = mybir.dt.float32
BF = mybir.dt.bfloat16
FR = mybir.dt.float32r
AF = mybir.ActivationFunctionType
ALU = mybir.AluOpType
AX = mybir.AxisListType

NCORES = 8
BPC = 2          # batches per core
C, HW = 256, 1024
MPHI, NN, DE = 16, 20, 300
KE = DE + 1      # 301 = DEMB + fused-bias row

LAST_EXEC_NS = None
LAST_RESULT = None


def _ksl(total, step=128):
    return [(o, min(step, total - o)) for o in range(0, total, step)]


def _fr(ap):
    return ap.bitcast(FR)


# ---------------------------------------------------------------------------
# weight-pack layouts (shared between host packing and kernel build)
# ---------------------------------------------------------------------------

class _PackAlloc:
    """First-fit strip allocator: blocks of equal width stack vertically in a
    128-row strip before opening a new column range."""

    def __init__(self):
        self.strips = []            # [col_off, width, used_rows]
        self.ncols = 0
        self.blocks = {}            # name -> (row, col, rows, cols)

    def add(self, name, rows, cols, stack=False):
        # PE matmul operands must sit at base partition 0 (they pair with
        # base-0 tiles); only non-matmul blocks may stack below other blocks.
        if stack:
            for s in self.strips:
                r = (s[2] + 31) // 32 * 32
                if s[1] == cols and r <= 64 and r + rows <= 128:
                    s[2] = r + rows
                    self.blocks[name] = (r, s[0], rows, cols)
                    return
        off = self.ncols
        self.ncols += cols
        self.strips.append([off, cols, rows])
        self.blocks[name] = (0, off, rows, cols)


def _mk_layout_p():
    a = _PackAlloc()
    for i in range(2):
        a.add(f"phiwT{i}", 128, MPHI)
    for i in range(2):
        a.add(f"globwT{i}", 128, MPHI)
    for i in range(2):
        a.add(f"win{i}", 128, 1)
    a.add("phib", MPHI, 1, stack=True)
    a.add("adj", NN, NN)
    return a


def _mk_layout_h():
    a = _PackAlloc()
    for i in range(2):
        a.add(f"gww{i}", 128, C)
    for i in range(4):
        a.add(f"fwT{i}", 128, C)
    return a


def _mk_layout_s():
    a = _PackAlloc()
    for nm, k in (("wq", DE), ("wk", DE), ("wve", KE), ("wo", DE)):
        for i, (o, s) in enumerate(_ksl(k)):
            a.add(f"{nm}{i}", s, DE)
    for i, (o, s) in enumerate(_ksl(DE)):
        a.add(f"gc1{i}", s, C)
    for i in range(2):
        a.add(f"gc2{i}", 128, C)
    for i, (o, s) in enumerate(_ksl(KE)):
        a.add(f"embTe{i}", s, NN)
    a.add("emb", NN, DE, stack=True)
    a.add("bo", 1, DE, stack=True)
    for nm, k in (("bq", DE), ("bk", DE)):
        for i, (o, s) in enumerate(_ksl(k)):
            a.add(f"{nm}{i}", s, 1, stack=True)
    return a


_LP = _mk_layout_p()
_LH = _mk_layout_h()
_LS = _mk_layout_s()


def _pack_p(inputs):
    f = lambda k: np.ascontiguousarray(inputs[k], dtype=np.float32)
    img = np.zeros((128, _LP.ncols), np.float32)

    def put(name, arr):
        r, c, rows, cols = _LP.blocks[name]
        img[r:r + rows, c:c + cols] = arr

    phiwT = f("phi_w").T
    globwT = f("glob_w").T
    for i, (o, s) in enumerate(_ksl(C)):
        put(f"phiwT{i}", phiwT[o:o + s])
        put(f"globwT{i}", globwT[o:o + s])
        put(f"win{i}", f("s2l_w")[:C].reshape(C, 1)[o:o + s])
    put("phib", f("phi_b").reshape(MPHI, 1))
    put("adj", f("adj"))
    return img


def _pack_h(inputs):
    f = lambda k: np.ascontiguousarray(inputs[k], dtype=np.float32)
    img = np.zeros((128, _LH.ncols), np.float32)

    def put(name, arr):
        r, c, rows, cols = _LH.blocks[name]
        img[r:r + rows, c:c + cols] = arr

    for i, (o, s) in enumerate(_ksl(C)):
        put(f"gww{i}", f("gw_w")[o:o + s])
    fwT = f("final_w").T
    for i, (o, s) in enumerate(_ksl(2 * C)):
        # rows 0:256 (the Wlg half) are negated: the kernel computes
        # -spiral^T, and (-Wlg) @ (-spiral^T) = Wlg @ spiral^T.
        blk = fwT[o:o + s]
        put(f"fwT{i}", -blk if i < 2 else blk)
    return img


def _pack_s(inputs):
    bf = mybir.dt.np(BF)
    f = lambda k: np.ascontiguousarray(inputs[k], dtype=np.float32)
    img = np.zeros((128, _LS.ncols), bf)

    def put(name, arr):
        r, c, rows, cols = _LS.blocks[name]
        img[r:r + rows, c:c + cols] = arr.astype(bf)

    wve = np.vstack([f("wv"), f("bv")[None, :]])
    embTe = np.vstack([f("emb").T, np.ones((1, NN), np.float32)])
    for nm, k, arr in (("wq", DE, f("wq")), ("wk", DE, f("wk")),
                       ("wve", KE, wve), ("wo", DE, f("wo")),
                       ("gc1", DE, f("gc1_w")), ("embTe", KE, embTe),
                       ("bq", DE, f("bq").reshape(DE, 1)),
                       ("bk", DE, f("bk").reshape(DE, 1))):
        for i, (o, s) in enumerate(_ksl(k)):
            put(f"{nm}{i}", arr[o:o + s])
    for i, (o, s) in enumerate(_ksl(C)):
        put(f"gc2{i}", f("gc2_w")[o:o + s])
    put("emb", f("emb"))
    put("bo", f("bo").reshape(1, DE))
    return img


# ---------------------------------------------------------------------------
# kernel build
# ---------------------------------------------------------------------------

def _build_nc():
    nc = bacc.Bacc()

    x_p = nc.declare_dram_parameter("x", [BPC, C * HW], FP, isOutput=False)
    out_p = nc.declare_dram_parameter("out", [BPC, C * HW], FP, isOutput=True)
    pp_p = nc.declare_dram_parameter("wpackP", [128, _LP.ncols], FP,
                                     isOutput=False)
    ph_p = nc.declare_dram_parameter("wpackH", [128, _LH.ncols], FP,
                                     isOutput=False)
    ps_p = nc.declare_dram_parameter("wpackS", [128, _LS.ncols], BF,
                                     isOutput=False)
    rscr = nc.dram_tensor("rscratch", [BPC, MPHI * HW], FP)

    with tile.TileContext(nc) as tc:
        with nc.allow_low_precision(reason="float32r/bf16 matmul feeds"), \
             ExitStack() as ctx:
            _body(ctx, tc, nc, x_p, out_p, pp_p, ph_p, ps_p, rscr)
    nc.finalize()
    return nc


def _body(ctx, tc, nc, x_p, out_p, pp_p, ph_p, ps_p, rscr):
    cw = ctx.enter_context(tc.tile_pool(name="cw", bufs=1))      # persistent
    sem = ctx.enter_context(tc.tile_pool(name="sem", bufs=1))    # semantic
    sm = ctx.enter_context(tc.tile_pool(name="sm", bufs=2))      # small/batch
    xm = ctx.enter_context(tc.tile_pool(name="xm", bufs=2))
    xvp = ctx.enter_context(tc.tile_pool(name="xvp", bufs=2))
    rp = ctx.enter_context(tc.tile_pool(name="rp", bufs=2))
    etp = ctx.enter_context(tc.tile_pool(name="etp", bufs=16))
    spp = ctx.enter_context(tc.tile_pool(name="spp", bufs=2))
    obp = ctx.enter_context(tc.tile_pool(name="obp", bufs=4))
    ps_w = ctx.enter_context(tc.tile_pool(name="ps_w", bufs=2, space="PSUM"))
    ps_x = ctx.enter_context(tc.tile_pool(name="ps_x", bufs=3, space="PSUM"))
    ps_t = ctx.enter_context(tc.tile_pool(name="ps_t", bufs=1, space="PSUM"))

    def mm(out, lhsT, rhs, start, stop):
        nc.tensor.matmul(out, _fr(lhsT), _fr(rhs), start=start, stop=stop)

    def mmb(out, lhsT, rhs, start, stop):
        nc.tensor.matmul(out, lhsT, rhs, start=start, stop=stop)

    # ---------------- constants + PE warmup + first loads ----------------
    ident = cw.tile([128, 128], FP, tag="ident")
    masks.make_identity(nc, ident[:])

    # p-state warmup: dummy transposes keep the PE continuously busy while
    # the first DMAs land, so real matmuls start at full clock.
    for i in range(12):
        wu = ps_t.tile([128, 128], FP, tag="ps_t", name=f"wu{i}")
        nc.tensor.transpose(_fr(wu[:]), _fr(ident[:]), _fr(ident[:]))

    packP = cw.tile([128, _LP.ncols], FP, tag="packP")
    nc.sync.dma_start(_fr(packP[:]), _fr(pp_p[:]))

    def wp(name):
        r, c, rows, cols = _LP.blocks[name]
        return packP[r:r + rows, c:c + cols]

    packH = cw.tile([128, _LH.ncols], FP, tag="packH")
    packS = cw.tile([128, _LS.ncols], BF, tag="packS")

    def wh(name):
        r, c, rows, cols = _LH.blocks[name]
        return packH[r:r + rows, c:c + cols]

    def ws(name):
        r, c, rows, cols = _LS.blocks[name]
        return packS[r:r + rows, c:c + cols]

    Me = [cw.tile([MPHI + 1, HW], FP, tag=f"Me{b}", name=f"Me{b}")
          for b in range(BPC)]
    for b in range(BPC):
        nc.gpsimd.memset(Me[b][MPHI:MPHI + 1, :], 1.0)
    one_row_b = cw.tile([1, NN], BF, tag="one_row_b")
    nc.gpsimd.memset(one_row_b[:], 1.0)
    ones20b = cw.tile([NN, 8], BF, tag="ones20b")
    nc.gpsimd.memset(ones20b[:], 1.0)
    inv20b = cw.tile([NN, 8], BF, tag="inv20b")
    nc.gpsimd.memset(inv20b[:], 1.0 / NN)

    x_mat = x_p[:].rearrange("b (c2 c q) -> b c c2 q", c2=2, c=128, q=HW)
    x_raw = x_p[:].rearrange("b (t q c) -> b q t c", t=8, q=128, c=C)
    out_v = out_p[:].rearrange("b (o2 o q) -> b o o2 q", o2=2, o=128, q=HW)
    r_st = rscr[:].rearrange("b (j q) -> b j q", j=MPHI)
    r_ld = rscr[:].rearrange("b (t p m) -> b p t m", t=8, p=128, m=MPHI)

    xmat = [None] * BPC
    xvt = [None] * BPC
    xvv = [None] * BPC

    def load_xmat(b):
        t = xm.tile([128, 2 * HW], FP, tag="xmat", name=f"xmat{b}")
        d = nc.sync.dma_start(_fr(t[:].rearrange("c (c2 q) -> c c2 q", c2=2)),
                              _fr(x_mat[b]))
        xmat[b] = t
        return d

    def load_xv(b):
        t = xvp.tile([128, 8 * (C + 1)], FP, tag="xv", name=f"xv{b}")
        v = t[:].rearrange("q (t c) -> q t c", t=8)
        d = nc.sync.dma_start(_fr(v[:, :, 0:C]), _fr(x_raw[b]))
        nc.gpsimd.memset(v[:, :, C:C + 1], 1.0)
        xvt[b] = t
        xvv[b] = v
        return d

    d_xmat0 = load_xmat(0)

    # adjacency normalisation early: its Sqrt table load must not land in
    # the middle of the exp stream.  adj_n = (d (x) d) * (adj + I)
    ah = sem.tile([NN, NN], FP, tag="ah")
    nc.gpsimd.tensor_add(ah[:], wp("adj"), ident[0:NN, 0:NN])
    r20 = sem.tile([NN, 1], FP, tag="r20")
    nc.vector.tensor_reduce(r20[:], ah[:], axis=AX.X, op=ALU.add)
    ir20 = sem.tile([NN, 1], FP, tag="ir20")
    nc.vector.reciprocal(ir20[:], r20[:])
    d20 = sem.tile([NN, 1], FP, tag="d20")
    nc.scalar.activation(d20[:], ir20[:], AF.Sqrt)
    ps = ps_t.tile([1, NN], FP, tag="ps_t")
    nc.tensor.transpose(ps[:], d20[:, 0:1], ident[0:NN, 0:NN])
    dT = sem.tile([1, NN], FP, tag="dT")
    nc.vector.tensor_copy(_fr(dT[:]), ps[:])
    ps = ps_t.tile([NN, NN], FP, tag="ps_t")
    mm(ps[:], dT[:], dT[:], start=True, stop=True)
    adjn = sem.tile([NN, NN], FP, tag="adjn")
    nc.vector.tensor_mul(adjn[:], ah[:], ps[:])
    ps = ps_t.tile([NN, NN], FP, tag="ps_t")
    nc.tensor.transpose(ps[:], adjn[:], ident[0:NN, 0:NN])
    adjnT = sem.tile([NN, NN], BF, tag="adjnT")
    nc.vector.tensor_copy(adjnT[:], ps[:])

    # ---------------- per-batch phases ----------------
    R = [None] * BPC
    negMm = [None] * BPC
    xpT = [None] * BPC
    ea = [None] * BPC
    fab = [None] * BPC
    ETb = [None] * BPC

    def phase_pre(b):
        """phi -> R (+spill), Dg, M rows, -rowmax."""
        xmv = xmat[b][:].rearrange("c (c2 q) -> c c2 q", c2=2)
        pphi = ps_w.tile([MPHI, HW], FP, tag="ps_w")
        for ki in range(2):
            for nh in range(2):
                mm(pphi[:, 512 * nh:512 * (nh + 1)], wp(f"phiwT{ki}"),
                   xmv[:, ki, 512 * nh:512 * (nh + 1)],
                   start=(ki == 0), stop=(ki == 1))
        Rb = rp.tile([MPHI, HW], FP, tag="R")
        nc.scalar.activation(_fr(Rb[:]), pphi[:], AF.Relu,
                             bias=wp("phib")[:, 0:1])
        d_rst = nc.gpsimd.dma_start(r_st[b], Rb[:])
        R[b] = Rb

        xmean = sm.tile([128, 2], FP, tag="xmean")
        for ki in range(2):
            nc.vector.tensor_reduce(xmean[:, ki:ki + 1], xmv[:, ki, :],
                                    axis=AX.X, op=ALU.add)
        pg = ps_t.tile([MPHI, 1], FP, tag="ps_t")
        for ki in range(2):
            mm(pg[:], wp(f"globwT{ki}"), xmean[:, ki:ki + 1],
               start=(ki == 0), stop=(ki == 1))
        # sigmoid via exp+recip (keeps the act table on the main set)
        eng = sm.tile([MPHI, 1], FP, tag="eng")
        nc.scalar.activation(eng[:], pg[:, 0:1], AF.Exp,
                             scale=float(-1.0 / HW))
        nc.gpsimd.tensor_scalar_add(eng[:], eng[:], 1.0)
        sm05 = sm.tile([MPHI, 1], FP, tag="sm05")
        nc.vector.reciprocal(sm05[:], eng[:])
        nc.gpsimd.tensor_scalar_add(sm05[:], sm05[:], -0.5)
        Dgb = sm.tile([MPHI, MPHI], FP, tag="Dg")
        nc.gpsimd.tensor_scalar(_fr(Dgb[:]), ident[0:MPHI, 0:MPHI],
                                sm05[:, 0:1], 0.5, op0=ALU.mult, op1=ALU.add)

        pm = ps_w.tile([MPHI, HW], FP, tag="ps_w")
        for nh in range(2):
            mm(pm[:, 512 * nh:512 * (nh + 1)], Dgb[:],
               Rb[:, 512 * nh:512 * (nh + 1)], start=True, stop=True)
        nc.vector.tensor_copy(_fr(Me[b][0:MPHI, :]), pm[:])
        Mmax = sm.tile([MPHI, 1], FP, tag="Mmax")
        nc.vector.tensor_reduce(Mmax[:], pm[:], axis=AX.X, op=ALU.max)
        nMm = sm.tile([MPHI, 1], FP, tag="negMm")
        nc.vector.tensor_scalar_mul(_fr(nMm[:]), Mmax[:], -1.0)
        negMm[b] = nMm
        return d_rst

    def phase_xpt(b):
        """x_phi reload (raw reshape), PE transposes, ub row."""
        xpa = sm.tile([128, 128], FP, tag="xpa")
        d_xpl = nc.gpsimd.dma_start(
            xpa[:].rearrange("p (t m) -> p t m", t=8), r_ld[b])
        xt = rp.tile([MPHI + 1, HW], FP, tag="xpT")
        for h in range(2):
            psx = ps_x.tile([MPHI, 512], FP, tag="ps_x")
            for j in range(4):
                t8 = 4 * h + j
                nc.tensor.transpose(_fr(psx[:, 128 * j:128 * (j + 1)]),
                                    _fr(xpa[:, MPHI * t8:MPHI * (t8 + 1)]),
                                    _fr(ident[:, :]))
            nc.vector.tensor_copy(_fr(xt[0:MPHI, 512 * h:512 * (h + 1)]),
                                  psx[:])
        pub = ps_w.tile([1, HW], FP, tag="ps_w")
        for nh in range(2):
            mm(pub[:, 512 * nh:512 * (nh + 1)], negMm[b][:, 0:1],
               xt[0:MPHI, 512 * nh:512 * (nh + 1)], start=True, stop=True)
        # nub row split across ACT and DVE (it sits on the critical path)
        nc.scalar.copy(_fr(xt[MPHI:MPHI + 1, 0:512]), pub[:, 0:512])
        nc.vector.tensor_copy(_fr(xt[MPHI:MPHI + 1, 512:HW]), pub[:, 512:HW])
        xpT[b] = xt
        return d_xpl

    def phase_ea(b):
        """softmax-over-pixels numerator (normalisation folded into fa)."""
        xmv = xmat[b][:].rearrange("c (c2 q) -> c c2 q", c2=2)
        pa = ps_w.tile([1, HW], FP, tag="ps_w")
        for ki in range(2):
            for nh in range(2):
                mm(pa[:, 512 * nh:512 * (nh + 1)], wp(f"win{ki}"),
                   xmv[:, ki, 512 * nh:512 * (nh + 1)],
                   start=(ki == 0), stop=(ki == 1))
        eab = rp.tile([1, HW], FP, tag="ea")
        sae = sm.tile([1, 1], FP, tag="sae")
        nc.scalar.activation(eab[:], pa[:], AF.Exp, accum_out=sae[:, 0:1])
        sar = sm.tile([1, 1], FP, tag="sar", name=f"sar{b}")
        nc.vector.reciprocal(sar[:], sae[:])
        ea[b] = eab
        return sar

    def phase_et(b):
        """E^T tiles [128, 1024] = exp(S^T - ub)."""
        ET = []
        for t8 in range(8):
            pst = ps_w.tile([128, HW], FP, tag="ps_w")
            for nh in range(2):
                mm(pst[:, 512 * nh:512 * (nh + 1)],
                   Me[b][:, 128 * t8:128 * (t8 + 1)],
                   xpT[b][:, 512 * nh:512 * (nh + 1)], start=True, stop=True)
            et = etp.tile([128, HW], FP, tag="et")
            nc.scalar.activation(_fr(et[:]), pst[:], AF.Exp)
            ET.append(et)
        ETb[b] = ET

    def phase_exv(b):
        """EXV per p-tile; -spiral; transpose into -spiral^T."""
        ET = ETb[b]
        spT = spp.tile([128, 2 * HW], FP, tag="spT")
        spTv = spT[:].rearrange("c (ch p) -> c ch p", ch=2)
        for pt in range(8):
            pe = ps_x.tile([128, C + 1], FP, tag="ps_x")
            for k in range(8):
                mm(pe[:], ET[k][:, 128 * pt:128 * (pt + 1)],
                   xvt[b][:, 257 * k:257 * k + 257],
                   start=(k == 0), stop=(k == 7))
            rd = sm.tile([128, 1], FP, tag="rd")
            nc.vector.reciprocal(rd[:], pe[:, C:C + 1])
            spr = sm.tile([128, C], FP, tag="spr")
            nc.vector.scalar_tensor_tensor(spr[:], pe[:, 0:C], rd[:, 0:1],
                                           xvv[b][:, pt, 0:C],
                                           op0=ALU.mult, op1=ALU.subtract)
            ptr = ps_x.tile([128, C], FP, tag="ps_x")
            for ch in range(2):
                nc.tensor.transpose(_fr(ptr[:, 128 * ch:128 * (ch + 1)]),
                                    _fr(spr[:, 128 * ch:128 * (ch + 1)]),
                                    _fr(ident[:, :]))
            nc.vector.tensor_copy(
                _fr(spTv[:, :, 128 * pt:128 * (pt + 1)]),
                ptr[:].rearrange("p (ch q) -> p ch q", ch=2))
        return spTv

    def phase_out(b, spTv, WlgT):
        xmv = xmat[b][:].rearrange("c (c2 q) -> c c2 q", c2=2)
        for ot in range(2):
            po = ps_w.tile([128, HW], FP, tag="ps_w")
            for nh in range(2):
                sl = slice(512 * nh, 512 * (nh + 1))
                for ct in range(2):
                    mm(po[:, sl], WlgT[ct][:, 128 * ot:128 * (ot + 1)],
                       spTv[:, ct, sl], start=(ct == 0), stop=False)
                mm(po[:, sl], fab[b][0:1, 128 * ot:128 * (ot + 1)],
                   ea[b][0:1, sl], start=False, stop=False)
                mm(po[:, sl], ident[:, :], xmv[:, ot, sl],
                   start=False, stop=True)
            for h in range(2):
                ob = obp.tile([128, 512], FP, tag="ob")
                nc.scalar.activation(_fr(ob[:]), po[:, 512 * h:512 * (h + 1)],
                                     AF.Relu)
                nc.sync.dma_start(
                    _fr(out_v[b][:, ot, 512 * h:512 * (h + 1)]), _fr(ob[:]))

    # ---------------- emission: batch 0 front, staged loads ----------------
    phase_pre(0)
    d_xp0 = phase_xpt(0)
    sar0 = phase_ea(0)
    phase_et(0)

    # bulk loads pushed behind the batch-0 R roundtrip on the DMA device
    d_xv0 = load_xv(0)
    add_dep_helper(d_xv0.ins, d_xp0.ins, sync=True, reason="dma order")
    d_xmat1 = load_xmat(1)
    add_dep_helper(d_xmat1.ins, d_xp0.ins, sync=True, reason="dma order")
    d_packS = nc.sync.dma_start(packS[:], ps_p[:])
    add_dep_helper(d_packS.ins, d_xmat1.ins, sync=True, reason="dma order")

    phase_pre(1)
    d_xp1 = phase_xpt(1)
    sar1 = phase_ea(1)
    phase_et(1)

    d_xv1 = load_xv(1)
    add_dep_helper(d_xv1.ins, d_xp1.ins, sync=True, reason="dma order")
    d_packH = nc.sync.dma_start(_fr(packH[:]), _fr(ph_p[:]))
    add_dep_helper(d_packH.ins, d_xp1.ins, sync=True, reason="dma order")

    # ---------------- semantic branch (batch independent, bf16) ----------
    def qt_like(wname, bname, tag):
        outs = []
        for mi, (mo, ms) in enumerate(_ksl(DE)):
            ps = ps_t.tile([ms, NN], FP, tag="ps_t")
            for ki, (ko, ks) in enumerate(_ksl(DE)):
                mmb(ps[:], ws(f"{wname}{ki}")[:, mo:mo + ms],
                    ws(f"embTe{ki}")[0:ks, :], start=(ki == 0), stop=(ki == 2))
            t = sem.tile([ms, NN], BF, tag=f"{tag}{mi}")
            nc.scalar.activation(t[:], ps[:], AF.Identity,
                                 bias=ws(f"{bname}{mi}")[:, 0:1])
            outs.append(t)
        return outs

    qT = qt_like("wq", "bq", "qT")
    kT = qt_like("wk", "bk", "kT")

    ps = ps_t.tile([NN, DE], FP, tag="ps_t")
    for ki in range(3):
        mmb(ps[:], ws(f"embTe{ki}"), ws(f"wve{ki}"),
            start=(ki == 0), stop=(ki == 2))
    v_sb = sem.tile([NN, DE], BF, tag="v_sb")
    nc.vector.tensor_copy(v_sb[:], ps[:])

    ps = ps_t.tile([NN, NN], FP, tag="ps_t")
    for ki in range(3):
        mmb(ps[:], qT[ki][:], kT[ki][:], start=(ki == 0), stop=(ki == 2))
    att_s = sem.tile([NN, NN], FP, tag="att_s")
    nc.scalar.activation(att_s[:], ps[:], AF.Identity,
                         scale=float(1.0 / np.sqrt(DE)))
    mx = sem.tile([NN, 1], FP, tag="mx")
    nc.vector.tensor_reduce(mx[:], att_s[:], axis=AX.X, op=ALU.max)
    negmx = sem.tile([NN, 1], FP, tag="negmx")
    nc.vector.tensor_scalar_mul(negmx[:], mx[:], -1.0)
    att_e = sem.tile([NN, NN], FP, tag="att_e")
    rs = sem.tile([NN, 1], FP, tag="rs")
    nc.scalar.activation(att_e[:], att_s[:], AF.Exp, bias=negmx[:, 0:1],
                         accum_out=rs[:, 0:1])
    rr = sem.tile([NN, 1], FP, tag="rr")
    nc.vector.reciprocal(rr[:], rs[:])
    att_n = sem.tile([NN, NN], FP, tag="att_n")
    nc.vector.tensor_scalar_mul(att_n[:], att_e[:], rr[:, 0:1])

    ps = ps_t.tile([NN, NN], FP, tag="ps_t")
    nc.tensor.transpose(ps[:], att_n[:], ident[0:NN, 0:NN])
    attT = sem.tile([NN, NN], BF, tag="attT")
    nc.vector.tensor_copy(attT[:], ps[:])
    ps = ps_t.tile([NN, DE], FP, tag="ps_t")
    mmb(ps[:], attT[:], v_sb[:], start=True, stop=True)
    av_sb = sem.tile([NN, DE], BF, tag="av_sb")
    nc.vector.tensor_copy(av_sb[:], ps[:])

    n1c = sem.tile([128, 3], BF, tag="n1c")
    for mi, (mo, ms) in enumerate(_ksl(DE)):
        ps = ps_t.tile([ms, 8], FP, tag="ps_t")
        mmb(ps[:], av_sb[:, mo:mo + ms], inv20b[:], start=True, stop=True)
        nc.vector.tensor_copy(n1c[0:ms, mi:mi + 1], ps[:, 0:1])

    ps = ps_t.tile([1, DE], FP, tag="ps_t")
    for ki, (ko, ks) in enumerate(_ksl(DE)):
        mmb(ps[:], n1c[0:ks, ki:ki + 1], ws(f"wo{ki}"),
            start=(ki == 0), stop=(ki == 2))
    n2 = sem.tile([1, DE], BF, tag="n2")
    nc.vector.tensor_add(n2[:], ws("bo"), ps[:])
    ps = ps_t.tile([NN, DE], FP, tag="ps_t")
    mmb(ps[:], one_row_b[:], n2[:], start=True, stop=True)
    ev_sb = sem.tile([NN, DE], FP, tag="ev_sb")
    nc.vector.tensor_add(ev_sb[:], ws("emb"), ps[:])

    evT = []
    for mi, (mo, ms) in enumerate(_ksl(DE)):
        ps = ps_t.tile([ms, NN], FP, tag="ps_t")
        nc.tensor.transpose(ps[:], ev_sb[:, mo:mo + ms], ident[0:NN, 0:NN])
        t = sem.tile([ms, NN], BF, tag=f"evT{mi}")
        nc.vector.tensor_copy(t[:], ps[:])
        evT.append(t)

    ps = ps_t.tile([NN, C], FP, tag="ps_t")
    for ki in range(3):
        mmb(ps[:], evT[ki][:], ws(f"gc1{ki}"), start=(ki == 0), stop=(ki == 2))
    t1 = sem.tile([NN, C], BF, tag="t1")
    nc.vector.tensor_copy(t1[:], ps[:])
    ps = ps_t.tile([NN, C], FP, tag="ps_t")
    mmb(ps[:], adjnT[:], t1[:], start=True, stop=True)
    g1 = sem.tile([NN, C], FP, tag="g1")
    nc.vector.tensor_scalar_max(g1[:], ps[:], 0.0)

    g1T = []
    for mi, (mo, ms) in enumerate(_ksl(C)):
        ps = ps_t.tile([ms, NN], FP, tag="ps_t")
        nc.tensor.transpose(ps[:], g1[:, mo:mo + ms], ident[0:NN, 0:NN])
        t = sem.tile([ms, NN], BF, tag=f"g1T{mi}")
        nc.vector.tensor_copy(t[:], ps[:])
        g1T.append(t)

    ps = ps_t.tile([NN, C], FP, tag="ps_t")
    for ki in range(2):
        mmb(ps[:], g1T[ki][:], ws(f"gc2{ki}"), start=(ki == 0), stop=(ki == 1))
    t2 = sem.tile([NN, C], BF, tag="t2")
    nc.vector.tensor_copy(t2[:], ps[:])
    ps = ps_t.tile([NN, C], FP, tag="ps_t")
    mmb(ps[:], adjnT[:], t2[:], start=True, stop=True)
    g2 = sem.tile([NN, C], BF, tag="g2")
    nc.vector.tensor_scalar_max(g2[:], ps[:], 0.0)

    reluG = sem.tile([128, 2], FP, tag="reluG")
    for cb in range(2):
        ps = ps_t.tile([128, 8], FP, tag="ps_t")
        mmb(ps[:], g2[:, 128 * cb:128 * (cb + 1)], ones20b[:],
            start=True, stop=True)
        nc.scalar.activation(_fr(reluG[:, cb:cb + 1]), ps[:, 0:1], AF.Relu)
    ps = ps_t.tile([1, C], FP, tag="ps_t")
    for cb in range(2):
        mm(ps[:], reluG[:, cb:cb + 1], wh(f"fwT{2 + cb}"),
           start=(cb == 0), stop=(cb == 1))
    fa = sem.tile([1, C], FP, tag="fa")
    nc.vector.tensor_copy(_fr(fa[:]), ps[:])

    WlgT = []
    for cb in range(2):
        ps = ps_t.tile([128, C], FP, tag="ps_t")
        for ki in range(2):
            mm(ps[:], wh(f"gww{ki}")[:, 128 * cb:128 * (cb + 1)],
               wh(f"fwT{ki}"), start=(ki == 0), stop=(ki == 1))
        t = sem.tile([128, C], FP, tag=f"WlgT{cb}")
        nc.vector.tensor_copy(_fr(t[:]), ps[:])
        WlgT.append(t)

    for b, sar in ((0, sar0), (1, sar1)):
        fb = sm.tile([1, C], FP, tag="fab", name=f"fab{b}")
        nc.vector.tensor_scalar_mul(_fr(fb[:]), fa[:], sar[0:1, 0:1])
        fab[b] = fb

    # ---------------- heavy tail: EXV + out per batch ----------------
    spTv0 = phase_exv(0)
    phase_out(0, spTv0, WlgT)
    spTv1 = phase_exv(1)
    phase_out(1, spTv1, WlgT)


# ---------------------------------------------------------------------------
# host driver
# ---------------------------------------------------------------------------

def _prep_shared(inputs):
    return {"wpackP": _pack_p(inputs), "wpackH": _pack_h(inputs),
            "wpackS": _pack_s(inputs)}


_NC_CACHE = {}


def kernel(**inputs):
    global LAST_EXEC_NS, LAST_RESULT
    if "nc" not in _NC_CACHE:
        _NC_CACHE["nc"] = _build_nc()
    nc = _NC_CACHE["nc"]

    x = np.ascontiguousarray(inputs["x"], dtype=np.float32)
    B = x.shape[0]
    shared = _prep_shared(inputs)
    in_maps = []
    for i in range(NCORES):
        m = dict(shared)
        m["x"] = np.ascontiguousarray(
            x[i * BPC:(i + 1) * BPC].reshape(BPC, C * HW))
        in_maps.append(m)

    trace = os.environ.get("KERNEL_TRACE", "0") == "1"
    res = run_bass_kernel_spmd(nc, in_maps, list(range(NCORES)), trace=trace)
    LAST_RESULT = res
    LAST_EXEC_NS = getattr(res, "exec_time_ns", None)

    out = np.empty((B, C, 32, 32), np.float32)
    for i in range(NCORES):
        out[i * BPC:(i + 1) * BPC] = res.results[i]["out"].reshape(BPC, C, 32, 32)
    return out
